# revision 12
# baseline (speedup 1.0000x reference)
"""Trainium2 Bass kernel for the ABNet 10-head MLP ensemble + dCBF QP problem.

Sharding: pure data-parallel over the batch axis (B=16384 -> 2048 per core,
8 cores). All per-sample math, including the closed-form 1-constraint QP, is
local to a core; weights are replicated; no collectives.

Per-core compute layout (feature-major, batch in the free dimension):
  xT   [4, BL]        x transposed  (moving operand of layer 1)
  h1   [2048, BL]     = relu(W1.T x) stored as 16 chunks [128, BL] bf16
  L2   x2b[e,b]       = relu(sum_d W2b[d,e] h1[d,b]) via PE, psum [128, 512]
  L3   z3b[c,b]       = sum_e W3b[e,c] x2b[e,b], accumulated in psum at
                        partition offset 32*bt (PE array tiling)
  QP epilogue on DVE/ACT in fp32 on [1, 512] rows, weighted head sum.

Matmuls run in bf16 (1 cycle/row on PE vs 4 for fp32) with fp32 PSUM
accumulation; all non-matmul math stays fp32.
"""

import numpy as np

import concourse.bass as bass
import concourse.bacc as bacc
import concourse.mybir as mybir
from concourse.tile import TileContext
from concourse.bass_utils import run_bass_kernel_spmd

F32 = mybir.dt.float32
BF16 = mybir.dt.bfloat16
AF = mybir.ActivationFunctionType
ALU = mybir.AluOpType
AX = mybir.AxisListType

OBS_X, OBS_Y, RADIUS = 40.0, 15.0, 6.0
PI = float(np.pi)
TWO_PI = 2.0 * PI

N_CORES = 8
H_FULL, B_FULL, F_FULL, D_FULL, C_FULL = 10, 16384, 4, 2048, 2
BL_FULL = B_FULL // N_CORES

P = 128


def build_nc(H=H_FULL, F=F_FULL, D=D_FULL, C=C_FULL, BL=BL_FULL, NT=512):
    """Build the single-core Bass graph (SPMD: same graph on all cores)."""
    ND = D // P          # contraction chunks (layer 2)
    NE = D // P          # output-feature chunks (layer 2) == L3 contraction
    NB = BL // NT        # batch tiles
    assert D % P == 0 and BL % NT == 0 and NB <= 4

    nc = bacc.Bacc(None, target_bir_lowering=False)

    x_e = nc.declare_dram_parameter("x", [BL, F], F32, isOutput=False)
    W1_e = nc.declare_dram_parameter("W1", [H, F, D], F32, isOutput=False)
    b1_e = nc.declare_dram_parameter("b1", [H, D], F32, isOutput=False)
    W21_e = nc.declare_dram_parameter("W21", [H, D, D], F32, isOutput=False)
    b21_e = nc.declare_dram_parameter("b21", [H, D], F32, isOutput=False)
    W22_e = nc.declare_dram_parameter("W22", [H, D, D], F32, isOutput=False)
    b22_e = nc.declare_dram_parameter("b22", [H, D], F32, isOutput=False)
    W31_e = nc.declare_dram_parameter("W31", [H, D, C], F32, isOutput=False)
    b31_e = nc.declare_dram_parameter("b31", [H, C], F32, isOutput=False)
    W32_e = nc.declare_dram_parameter("W32", [H, D, C], F32, isOutput=False)
    b32_e = nc.declare_dram_parameter("b32", [H, C], F32, isOutput=False)
    wt_e = nc.declare_dram_parameter("wt", [H], F32, isOutput=False)
    mean_e = nc.declare_dram_parameter("mean", [F], F32, isOutput=False)
    std_e = nc.declare_dram_parameter("std", [F], F32, isOutput=False)
    out_e = nc.declare_dram_parameter("out", [BL, C], F32, isOutput=True)

    with TileContext(nc) as tc, tc.tile_pool(name="cp", bufs=1) as cp:
        # persistent per-sample rows + small constants
        def crow(tagname):
            return cp.tile([1, BL], F32, tag=tagname, name=tagname)

        xTb = cp.tile([F, BL], BF16, tag="xTb", name="xTb")
        bar16, bdot4, Lf2b = crow("bar16"), crow("bdot4"), crow("Lf2b")
        G0, G1, invGG = crow("G0"), crow("G1"), crow("invGG")
        outacc0, outacc1 = crow("outacc0"), crow("outacc1")
        wrow = cp.tile([1, H], F32, tag="wrow", name="wrow")

        # ------------- preamble (scratch pool, freed afterwards) -----------
        with tc.tile_pool(name="pre", bufs=1) as pre:
            def prow(tagname):
                return pre.tile([1, BL], F32, tag=tagname, name=tagname)

            xT = pre.tile([F, BL], F32, tag="xT", name="xT")
            nc.sync.dma_start(out=xT, in_=x_e[:, :].rearrange("b f -> f b"))
            nc.vector.tensor_copy(xTb, xT)

            stdR = pre.tile([1, F], F32, tag="stdR", name="stdR")
            nc.sync.dma_start(out=stdR, in_=std_e[None, :])
            meanR = pre.tile([1, F], F32, tag="meanR", name="meanR")
            nc.sync.dma_start(out=meanR, in_=mean_e[None, :])

            # un-normalized state rows x0[f] = x[:,f]*std[f] + mean[f]
            x0rows = []
            for f in range(F):
                xr = prow(f"x0r{f}")
                nc.sync.dma_start(
                    out=xr, in_=x_e[:, f:f + 1].rearrange("b one -> one b")
                )
                # two ops (not one fused mult+add): each engine instruction
                # only has budget for ~2 distinct semaphore waits, and xr,
                # stdR, meanR arrive via three different DMAs
                nc.vector.tensor_scalar(xr, xr, stdR[:, f:f + 1], None, op0=ALU.mult)
                nc.vector.tensor_scalar(xr, xr, meanR[:, f:f + 1], None, op0=ALU.add)
                x0rows.append(xr)
            px, py, th, v = x0rows

            # sin with range reduction into [-pi, pi]; sa/sb are ping-pong.
            # No mod on DVE: two rounds of conditional +-2pi handle |arg|<5pi,
            # ample for theta ~ N(mean, 1) with |mean| ~ O(1).
            def sin_reduced(out_t, arg_ap, sa, sb):
                nc.vector.tensor_scalar(sa, arg_ap, 0.0, None, op0=ALU.add)
                for _ in range(2):
                    nc.vector.tensor_scalar(sb, sa, PI, None, op0=ALU.is_gt)
                    nc.vector.scalar_tensor_tensor(
                        sa, sb, -TWO_PI, sa, op0=ALU.mult, op1=ALU.add
                    )
                    nc.vector.tensor_scalar(sb, sa, -PI, None, op0=ALU.is_lt)
                    nc.vector.scalar_tensor_tensor(
                        sa, sb, TWO_PI, sa, op0=ALU.mult, op1=ALU.add
                    )
                nc.scalar.activation(out_t, sa, AF.Sin)

            sa, sb = prow("sa"), prow("sb")
            st, ct = prow("st"), prow("ct")
            sin_reduced(st, th, sa, sb)
            thc = prow("thc")
            nc.vector.tensor_scalar(thc, th, PI / 2.0, None, op0=ALU.add)
            sin_reduced(ct, thc, sa, sb)

            dx, dy = prow("dx"), prow("dy")
            nc.vector.tensor_scalar(dx, px, -OBS_X, None, op0=ALU.add)
            nc.vector.tensor_scalar(dy, py, -OBS_Y, None, op0=ALU.add)
            vst, vct = prow("vst"), prow("vct")
            nc.vector.tensor_mul(vst, v, st)
            nc.vector.tensor_mul(vct, v, ct)

            # bar16 = 16*(dx^2 + dy^2 - R^2)
            nc.vector.tensor_mul(sa, dx, dx)
            nc.vector.tensor_mul(sb, dy, dy)
            nc.vector.tensor_add(sa, sa, sb)
            nc.vector.tensor_scalar(
                bar16, sa, -(RADIUS * RADIUS), 16.0, op0=ALU.add, op1=ALU.mult
            )

            # bdot4 = 8*(dx*vct + dy*vst)
            nc.vector.tensor_mul(sa, dx, vct)
            nc.vector.tensor_mul(sb, dy, vst)
            nc.vector.tensor_add(sa, sa, sb)
            nc.vector.tensor_scalar(bdot4, sa, 8.0, None, op0=ALU.mult)

            # Lf2b = 2*v^2 = Square(v * sqrt(2))
            nc.scalar.activation(Lf2b, v, AF.Square, scale=float(np.sqrt(2.0)))

            # G0 = 2*(dx*vst - dy*vct); G1 = -2*(dx*ct + dy*st)
            nc.vector.tensor_mul(sa, dx, vst)
            nc.vector.tensor_mul(sb, dy, vct)
            nc.vector.tensor_sub(sa, sa, sb)
            nc.vector.tensor_scalar(G0, sa, 2.0, None, op0=ALU.mult)
            nc.vector.tensor_mul(sa, dx, ct)
            nc.vector.tensor_mul(sb, dy, st)
            nc.vector.tensor_add(sa, sa, sb)
            nc.vector.tensor_scalar(G1, sa, -2.0, None, op0=ALU.mult)

            nc.vector.tensor_mul(sa, G0, G0)
            nc.vector.tensor_mul(sb, G1, G1)
            nc.vector.tensor_add(sa, sa, sb)
            nc.vector.reciprocal(invGG, sa)

            # softmax over wt -> wrow [1, H]
            wt_row = pre.tile([1, H], F32, tag="wt_row", name="wt_row")
            nc.sync.dma_start(out=wt_row, in_=wt_e[None, :])
            wred = pre.tile([1, 1], F32, tag="wred", name="wred")
            nc.vector.reduce_max(wred, wt_row, axis=AX.X)
            nwmax = pre.tile([1, 1], F32, tag="nwmax", name="nwmax")
            nc.vector.tensor_scalar(nwmax, wred, -1.0, None, op0=ALU.mult)
            wexp = pre.tile([1, H], F32, tag="wexp", name="wexp")
            nc.scalar.activation(wexp, wt_row, AF.Exp, bias=nwmax)
            nc.vector.reduce_sum(wred, wexp, axis=AX.X)
            winv = pre.tile([1, 1], F32, tag="winv", name="winv")
            nc.vector.reciprocal(winv, wred)
            nc.vector.tensor_scalar(wrow, wexp, winv, None, op0=ALU.mult)

            nc.vector.memset(outacc0, 0.0)
            nc.vector.memset(outacc1, 0.0)

        # ------------- main pools + head loop ------------------------------
        with (
            tc.tile_pool(name="hw", bufs=2) as hp,      # per-head small tensors
            tc.tile_pool(name="wst", bufs=2) as wsp,    # fp32 weight staging
            tc.tile_pool(name="wb", bufs=3) as wbp,     # bf16 weight blocks
            tc.tile_pool(name="h1p", bufs=1) as h1p,
            tc.tile_pool(name="xap", bufs=3) as xap,
            tc.tile_pool(name="ep", bufs=8) as ep,     # epilogue scratch
            tc.tile_pool(name="ps", bufs=3, space="PSUM") as psp,
            tc.tile_pool(name="accp", bufs=2, space="PSUM") as accp,
        ):
            for h in range(H):
                # per-head small tensors
                w1t = hp.tile([F, D], F32, tag="w1t", name=f"w1t_{h}", bufs=1)
                nc.sync.dma_start(out=w1t, in_=W1_e[h])
                w1tb = hp.tile([F, D], BF16, tag="w1tb", name=f"w1tb_{h}")
                nc.vector.tensor_copy(w1tb, w1t)

                b1t = hp.tile([P, ND], F32, tag="b1t", name=f"b1t_{h}")
                nc.sync.dma_start(
                    out=b1t, in_=b1_e[h].rearrange("(dc p) -> p dc", p=P)
                )
                b21t = hp.tile([P, NE], F32, tag="b21t", name=f"b21t_{h}")
                nc.sync.dma_start(
                    out=b21t, in_=b21_e[h].rearrange("(ec p) -> p ec", p=P)
                )
                b22t = hp.tile([P, NE], F32, tag="b22t", name=f"b22t_{h}")
                nc.sync.dma_start(
                    out=b22t, in_=b22_e[h].rearrange("(ec p) -> p ec", p=P)
                )

                w31s = hp.tile([P, NE * C], F32, tag="w31s", name=f"w31s_{h}")
                nc.sync.dma_start(
                    out=w31s.rearrange("p (ec c) -> p ec c", c=C),
                    in_=W31_e[h].rearrange("(ec p) c -> p ec c", p=P),
                )
                w31t = hp.tile([P, NE * C], BF16, tag="w31t", name=f"w31t_{h}")
                nc.vector.tensor_copy(w31t, w31s)
                w32s = hp.tile([P, NE * C], F32, tag="w32s", name=f"w32s_{h}")
                nc.sync.dma_start(
                    out=w32s.rearrange("p (ec c) -> p ec c", c=C),
                    in_=W32_e[h].rearrange("(ec p) c -> p ec c", p=P),
                )
                w32t = hp.tile([P, NE * C], BF16, tag="w32t", name=f"w32t_{h}")
                nc.vector.tensor_copy(w32t, w32s)

                b31R = hp.tile([1, C], F32, tag="b31R", name=f"b31R_{h}")
                nc.sync.dma_start(out=b31R, in_=b31_e[h][None, :])
                b32R = hp.tile([1, C], F32, tag="b32R", name=f"b32R_{h}")
                nc.sync.dma_start(out=b32R, in_=b32_e[h][None, :])

                # ---- layer 1: h1[d, b] = relu(W1.T x) ----
                h1 = [
                    h1p.tile([P, BL], BF16, tag=f"h1_{dc}", name=f"h1_{h}_{dc}")
                    for dc in range(ND)
                ]
                for dc in range(ND):
                    for bt in range(NB):
                        ps1 = psp.tile([P, NT], F32, tag="mm", name=f"ps1_{h}_{dc}_{bt}")
                        nc.tensor.matmul(
                            ps1,
                            w1tb[:, dc * P:(dc + 1) * P],
                            xTb[:, bt * NT:(bt + 1) * NT],
                            start=True,
                            stop=True,
                        )
                        nc.scalar.activation(
                            h1[dc][:, bt * NT:(bt + 1) * NT],
                            ps1,
                            AF.Relu,
                            bias=b1t[:, dc:dc + 1],
                        )

                # ---- layers 2+3 ----
                # Per branch one psum bank; batch-tile accumulators live at
                # partition offsets 0/32/64/96 (PE-array tile positions).
                acc31 = accp.tile([128, NT], F32, tag="acc31", name=f"acc31_{h}")
                acc32 = accp.tile([128, NT], F32, tag="acc32", name=f"acc32_{h}")
                accs = (acc31, acc32)
                for e in range(NE):
                    wst21 = wsp.tile([P, D], F32, tag="wst", name=f"wst21_{h}_{e}")
                    nc.sync.dma_start(
                        out=wst21.rearrange("p (dc j) -> p dc j", j=P),
                        in_=W21_e[h][:, e * P:(e + 1) * P].rearrange(
                            "(dc p) j -> p dc j", p=P
                        ),
                    )
                    wb21 = wbp.tile([P, D], BF16, tag="wb", name=f"wb21_{h}_{e}")
                    nc.vector.tensor_copy(wb21, wst21)

                    wst22 = wsp.tile([P, D], F32, tag="wst", name=f"wst22_{h}_{e}")
                    nc.sync.dma_start(
                        out=wst22.rearrange("p (dc j) -> p dc j", j=P),
                        in_=W22_e[h][:, e * P:(e + 1) * P].rearrange(
                            "(dc p) j -> p dc j", p=P
                        ),
                    )
                    wb22 = wbp.tile([P, D], BF16, tag="wb", name=f"wb22_{h}_{e}")
                    nc.scalar.copy(wb22, wst22)

                    for br, (wb, b2t, w3t) in enumerate(
                        ((wb21, b21t, w31t), (wb22, b22t, w32t))
                    ):
                        for bt in range(NB):
                            ps2 = psp.tile(
                                [P, NT], F32, tag="mm",
                                name=f"ps2_{h}_{e}_{br}_{bt}",
                            )
                            for dc in range(ND):
                                nc.tensor.matmul(
                                    ps2,
                                    wb[:, dc * P:(dc + 1) * P],
                                    h1[dc][:, bt * NT:(bt + 1) * NT],
                                    start=(dc == 0),
                                    stop=(dc == ND - 1),
                                )
                            xa = xap.tile(
                                [P, NT], BF16, tag="xa",
                                name=f"xa_{h}_{e}_{br}_{bt}",
                            )
                            nc.scalar.activation(
                                xa, ps2, AF.Relu, bias=b2t[:, e:e + 1]
                            )
                            sl = 32 * bt
                            nc.tensor.matmul(
                                accs[br][sl:sl + 2, :],
                                w3t[:, C * e:C * (e + 1)],
                                xa,
                                start=(e == 0),
                                stop=(e == NE - 1),
                                skip_group_check=True,
                                tile_position=(0, sl),
                            )

                # ---- QP epilogue + weighted head accumulation ----
                for bt in range(NB):
                    bs = slice(bt * NT, (bt + 1) * NT)
                    sl = 32 * bt

                    def et(tagname):
                        return ep.tile([1, NT], F32, tag="eps",
                                       name=f"{tagname}_{h}_{bt}")

                    # Compute engines can only start at partitions 0/32/64/96,
                    # so copy the [2, NT] psum slice to SBUF (legal, starts at
                    # sl) and DMA row 1 down to a partition-0 tile.
                    t31 = ep.tile([2, NT], F32, tag="t2", name=f"t31_{h}_{bt}", bufs=3)
                    nc.scalar.copy(t31, acc31[sl:sl + 2, :])
                    t32 = ep.tile([2, NT], F32, tag="t2", name=f"t32_{h}_{bt}", bufs=3)
                    nc.scalar.copy(t32, acc32[sl:sl + 2, :])
                    z31_1 = et("z31_1")
                    nc.sync.dma_start(out=z31_1, in_=t31[1:2, :])
                    z32_1 = et("z32_1")
                    nc.sync.dma_start(out=z32_1, in_=t32[1:2, :])

                    s0 = et("s0")
                    nc.scalar.activation(
                        s0, t32[0:1, :], AF.Sigmoid, bias=b32R[:, 0:1]
                    )
                    s1 = et("s1")
                    nc.scalar.activation(s1, z32_1, AF.Sigmoid, bias=b32R[:, 1:2])
                    x31_0 = et("x31_0")
                    nc.vector.tensor_scalar(
                        x31_0, t31[0:1, :], b31R[:, 0:1], None, op0=ALU.add
                    )
                    x31_1 = et("x31_1")
                    nc.vector.tensor_scalar(x31_1, z31_1, b31R[:, 1:2], None, op0=ALU.add)

                    ssum = et("ssum")
                    nc.vector.tensor_add(ssum, s0, s1)
                    sprod = et("sprod")
                    nc.vector.tensor_mul(sprod, s0, s1)

                    # h_rhs = Lf2b + ssum*bdot4 + sprod*bar16
                    nc.vector.tensor_mul(ssum, ssum, bdot4[:, bs])
                    nc.vector.tensor_mul(sprod, sprod, bar16[:, bs])
                    nc.vector.tensor_add(ssum, ssum, sprod)
                    hrhs = et("hrhs")
                    nc.vector.tensor_add(hrhs, ssum, Lf2b[:, bs])

                    # Gu = G0*x31_0 + G1*x31_1
                    gu0 = et("gu0")
                    nc.vector.tensor_mul(gu0, G0[:, bs], x31_0)
                    gu1 = et("gu1")
                    nc.vector.tensor_mul(gu1, G1[:, bs], x31_1)
                    nc.vector.tensor_add(gu0, gu0, gu1)

                    # lam = relu(Gu - hrhs) * invGG
                    nc.vector.tensor_sub(gu0, gu0, hrhs)
                    nc.vector.tensor_scalar_max(gu0, gu0, 0.0)
                    lam = et("lam")
                    nc.vector.tensor_mul(lam, gu0, invGG[:, bs])

                    # u_c = x31_c - lam*G_c ; outacc_c += w[h]*u_c
                    lg0 = et("lg0")
                    nc.vector.tensor_mul(lg0, lam, G0[:, bs])
                    nc.vector.tensor_sub(x31_0, x31_0, lg0)
                    nc.vector.scalar_tensor_tensor(
                        outacc0[:, bs], x31_0, wrow[:, h:h + 1], outacc0[:, bs],
                        op0=ALU.mult, op1=ALU.add,
                    )
                    lg1 = et("lg1")
                    nc.vector.tensor_mul(lg1, lam, G1[:, bs])
                    nc.vector.tensor_sub(x31_1, x31_1, lg1)
                    nc.vector.scalar_tensor_tensor(
                        outacc1[:, bs], x31_1, wrow[:, h:h + 1], outacc1[:, bs],
                        op0=ALU.mult, op1=ALU.add,
                    )

            # ---------------- output ---------------------------------------
            nc.sync.dma_start(
                out=out_e[:, 0:1].rearrange("b one -> one b"), in_=outacc0
            )
            nc.sync.dma_start(
                out=out_e[:, 1:2].rearrange("b one -> one b"), in_=outacc1
            )

    nc.finalize()
    return nc


_nc_cache = None


def _get_nc():
    global _nc_cache
    if _nc_cache is None:
        _nc_cache = build_nc()
    return _nc_cache


_WEIGHT_NAMES = (
    "W1", "b1", "W21", "b21", "W22", "b22",
    "W31", "b31", "W32", "b32", "wt", "mean", "std",
)


def kernel(**inputs) -> np.ndarray:
    x = np.ascontiguousarray(np.asarray(inputs["x"], dtype=np.float32))
    rep = {
        k: np.ascontiguousarray(np.asarray(inputs[k], dtype=np.float32))
        for k in _WEIGHT_NAMES
    }
    nc = _get_nc()
    in_maps = []
    for i in range(N_CORES):
        m = dict(rep)
        m["x"] = np.ascontiguousarray(x[i * BL_FULL:(i + 1) * BL_FULL])
        in_maps.append(m)
    res = run_bass_kernel_spmd(nc, in_maps, core_ids=list(range(N_CORES)))
    outs = [np.asarray(res.results[i]["out"]) for i in range(N_CORES)]
    return np.concatenate(outs, axis=0).astype(np.float32)
